# revision 68
# baseline (speedup 1.0000x reference)
"""TRN2 Bass kernel for GQA MultiHeadAttention (B=2, S=2048, D=2048, 16 q-heads,
4 kv-heads, d_k=128) with QK-RMSNorm + interleaved RoPE + causal softmax + out-proj.

Sharding: 8 cores = (batch b in {0,1}) x (kv-head group g in {0..3}).
Each core computes its 4 q-heads' attention for its batch and a partial
out-projection y.T = Wo_g @ attn_out_g.T  [2048(e) x 2048(s)].
Host sums the 4 partials per batch (bf16 partials) and transposes.

Device layouts (all "head-dim on partitions", so no on-device transposes):
  xT   [d=16x128, s]        (moving operand of all projections)
  qT/kT [c=128, s]          RoPE'd + normalized, bf16; c0/rms_k folded into kT
  V    [s-in-block=128, 16 blocks, c=128]  built via PE transpose
  scores ST [j=128, i<=512] via matmul(lhsT=kT-block, rhs=qT-tile), j-blocks
  paired two-per-psum-tile so one ACT exp covers 1024 columns.
  P = exp(ST) masked; AO.T [c, i] = sum_j V.T P
No softmax max-subtraction: RMSNorm bounds |score| <= sqrt(128), exp is safe.
RoPE pair-interleave is folded into a host-side row permutation of Wq/Wk.

Perf structure:
  - "ones-matrix" matmuls (lhsT = value replicated over 128 columns) compute a
    partition-dim sum AND broadcast it to all 128 partitions in one op; the
    1/x and 1/sqrt(x) follow as exp(-ln x) on the ACT engine (one table set:
    Ln/Exp/Identity/Square), so no vector reciprocals and no extra broadcasts.
  - phase-1 2-deep and phase-2 one-pair-ahead software pipelines keep PE fed.
  - softmax denominators: P partial-summed on DVE (pairs -> quads -> total,
    diagonal pairs with packed-half offsets), ONE broadcast-sum matmul per
    (h, i-tile); the tail Ln/Exp is split across jobs so the single psum
    bank frees early and AV never waits on tail ACTs.
  - the out-projection is interleaved into the attention job stream one
    (eb, st) chunk per job (chunks become ready as each i-tile's aon lands),
    with bias-adds alternating DVE/ACT; only a small rump runs after.
  - output in bf16; host sums the 4 per-group partials per batch in f32.
"""
import sys
import numpy as np
import ml_dtypes

sys.path.insert(0, "/opt/trn_rl_repo")

import concourse.bass as bass  # noqa: E402
import concourse.tile as tile  # noqa: E402
from concourse import mybir  # noqa: E402
from concourse.bass_utils import run_bass_kernel_spmd  # noqa: E402

F32 = mybir.dt.float32
BF16 = mybir.dt.bfloat16
AF = mybir.ActivationFunctionType

P = 128
S = 2048
D = 2048
DK = 128
NH_LOC = 4          # q heads per core
NC_CHUNKS = D // P  # 16 contraction chunks
N_STILE = 4         # s-tiles of 512
STILE = 512
NJB = S // P        # 16 j/s blocks of 128
EPS = 1e-8
C0 = 1.0 / np.sqrt(DK)

_BF = ml_dtypes.bfloat16


_NO_SPLIT_OPCODES = {"UnconditionalBranch", "Call", "RegisterMove", "EventSemaphore"}
_WAIT_LIMIT = {}  # hw instruction structs take a single sync wait


def _split_excess_waits(nc):
    """Walrus codegen allows only 1-2 sync waits per instruction struct; Tile
    can emit more. Move excess waits onto same-engine NoOps inserted before."""
    import bass_rust
    counter = [0]
    for fn in nc.m.functions:
        for blk in fn.blocks:
            out = []
            changed = False
            for inst in blk.instructions:
                si = inst.sync_info
                limit = _WAIT_LIMIT.get(inst.opcode, 1)
                if (si is not None and len(si.on_wait) > limit
                        and inst.opcode not in _NO_SPLIT_OPCODES):
                    waits = list(si.on_wait)
                    for w in waits[:-limit]:
                        counter[0] += 1
                        nop = bass_rust.InstNoOp(
                            name=f"I-wsplit-{counter[0]}", ins=[], outs=[])
                        nop.engine = inst.engine
                        nop.sync_info = mybir.SyncInfo(on_wait=[w], on_update=[])
                        out.append(nop)
                    inst.sync_info = mybir.SyncInfo(
                        on_wait=waits[-limit:], on_update=list(si.on_update))
                    changed = True
                out.append(inst)
            if changed:
                blk.instructions = out
    return counter[0]


def _build_program(split=True):
    nc = bass.Bass()

    ext = {}

    def inp(name, shape, dt):
        ext[name] = nc.declare_dram_parameter(name, list(shape), dt, isOutput=False)
        return ext[name]

    xt4 = inp("xt4", (N_STILE, NC_CHUNKS, P, STILE), BF16)
    wq = inp("wq", (NC_CHUNKS, P, NH_LOC * DK), BF16)
    wk = inp("wk", (NC_CHUNKS, P, DK), BF16)
    wv = inp("wv", (NC_CHUNKS, P, DK), BF16)
    wo = inp("wo", (NH_LOC, P, D), BF16)
    winvq = inp("winvq", (P, P), BF16)   # 1/w_q^2 replicated over 128 cols
    winvk = inp("winvk", (P, P), BF16)
    id128 = inp("id128", (P, P), BF16)   # identity for PE transpose
    taba = inp("taba", (P, S), BF16)     # [cos; cos]
    tabb = inp("tabb", (P, S), BF16)     # [-sin; sin]
    maskt = inp("maskt", (P, P), BF16)   # lower-tri for the diagonal 128-block
    bq = inp("bq", (P, NH_LOC), F32)
    bk = inp("bk", (P, 1), F32)
    boeff = inp("boeff", (P, NJB), F32)
    yT = nc.declare_dram_parameter("yT", [D, S], BF16, isOutput=True)

    from contextlib import ExitStack

    with tile.TileContext(nc) as tc, ExitStack() as top:
        const = top.enter_context(tc.tile_pool(name="const", bufs=1))

        wq_sb = const.tile([P, NC_CHUNKS, NH_LOC * DK], BF16, tag="wq")
        wk_sb = const.tile([P, NC_CHUNKS, DK], BF16, tag="wk")
        wv_sb = const.tile([P, NC_CHUNKS, DK], BF16, tag="wv")
        winvq_sb = const.tile([P, P], BF16, tag="winvq")
        winvk_sb = const.tile([P, P], BF16, tag="winvk")
        id_sb = const.tile([P, P], BF16, tag="id128")
        taba_sb = const.tile([P, S], BF16, tag="taba")
        tabb_sb = const.tile([P, S], BF16, tag="tabb")
        mask_sb = const.tile([P, P], BF16, tag="mask")
        bq_sb = const.tile([P, NH_LOC], F32, tag="bq")
        bk_sb = const.tile([P, 1], F32, tag="bk")
        bo_sb = const.tile([P, NJB], F32, tag="bo")
        wo_sb = const.tile([P, NH_LOC, D], BF16, tag="wo")

        ones_mat = const.tile([P, P], BF16, tag="onesm")
        nc.vector.memset(ones_mat[:], 1.0)
        eps_sb = const.tile([P, 1], F32, tag="eps")
        nc.vector.memset(eps_sb[:], EPS)
        lnc0_sb = const.tile([P, 1], F32, tag="lnc0")
        nc.vector.memset(lnc0_sb[:], float(np.log(C0)))

        # persistent activation tensors
        qhat = const.tile([P, NH_LOC, S], BF16, tag="qhat")   # [c, h, s]
        khat = const.tile([P, S], BF16, tag="khat")           # [c, s], c0/rms_k folded
        vsb = const.tile([P, NJB, DK], BF16, tag="v")         # [s%128, block, c]
        aon = const.tile([P, NH_LOC, S], BF16, tag="aon")     # [c, h, i]

        # ------- Phase 1: projections + RMS + RoPE, software-pipelined -------
        with ExitStack() as ph1:
            xp = ph1.enter_context(tc.tile_pool(name="xp", bufs=2))
            t1p = ph1.enter_context(tc.tile_pool(name="t1p", bufs=4))
            rp = ph1.enter_context(tc.tile_pool(name="rp", bufs=3))
            pbp = ph1.enter_context(tc.tile_pool(name="pbp", bufs=3))
            ps1 = ph1.enter_context(tc.tile_pool(name="ps1", bufs=3, space="PSUM"))
            psl = ph1.enter_context(tc.tile_pool(name="psl", bufs=2, space="PSUM"))
            pst = ph1.enter_context(tc.tile_pool(name="pst", bufs=1, space="PSUM"))

            # startup-critical loads first: q weights + first x tile, interleaved
            xt0 = xp.tile([P, NC_CHUNKS, STILE], BF16, tag="xt", name="xt0")
            nc.sync.dma_start(wq_sb[:, 0:1, :], wq[0:1].rearrange("c p m -> p c m"))
            nc.sync.dma_start(xt0[:, 0:1, :], xt4[0, 0:1].rearrange("c p s -> p c s"))
            for g4 in range(0, NC_CHUNKS, 4):
                sl4 = slice(max(g4, 1), g4 + 4)
                nc.sync.dma_start(wq_sb[:, sl4, :], wq[sl4].rearrange("c p m -> p c m"))
                nc.sync.dma_start(xt0[:, sl4, :],
                                  xt4[0, sl4].rearrange("c p s -> p c s"))
            nc.sync.dma_start(bq_sb[:], bq[:])
            nc.sync.dma_start(winvq_sb[:], winvq[:])
            nc.sync.dma_start(taba_sb[:], taba[:])
            nc.sync.dma_start(tabb_sb[:], tabb[:])
            for g4 in range(0, NC_CHUNKS, 4):
                sl4 = slice(g4, g4 + 4)
                nc.sync.dma_start(wk_sb[:, sl4, :], wk[sl4].rearrange("c p m -> p c m"))
                nc.sync.dma_start(wv_sb[:, sl4, :], wv[sl4].rearrange("c p m -> p c m"))
            nc.sync.dma_start(winvk_sb[:], winvk[:])
            nc.sync.dma_start(id_sb[:], id128[:])
            nc.sync.dma_start(bk_sb[:], bk[:])
            # xt(1) queued here so it isn't stuck behind st0's swap DMAs
            xt1 = xp.tile([P, NC_CHUNKS, STILE], BF16, tag="xt", name="xt1")
            for g4 in range(0, NC_CHUNKS, 4):
                sl4 = slice(g4, g4 + 4)
                nc.sync.dma_start(xt1[:, sl4, :],
                                  xt4[1, sl4].rearrange("c p s -> p c s"))

            # --- pipeline stages (emission order = per-engine issue order) ---
            def stage_A(oi, xt, first=False):
                """projection matmuls + bias ACT + square ACT + swap DMAs"""
                is_q = oi != "k"
                pt = ps1.tile([P, STILE], F32, tag="proj")
                for ch in range(NC_CHUNKS):
                    lw = wq_sb[:, ch, bass.ts(oi, DK)] if is_q else wk_sb[:, ch, :]
                    nc.tensor.matmul(pt[:], lw, xt[:, ch, :],
                                     start=(ch == 0), stop=(ch == NC_CHUNKS - 1))
                bias_ap = bq_sb[:, oi : oi + 1] if is_q else bk_sb[:, 0:1]
                qf = t1p.tile([P, STILE], F32, tag="qf")
                nc.scalar.activation(qf[:], pt[:], AF.Identity, bias=bias_ap)
                sq = t1p.tile([P, STILE], BF16, tag="sq")
                nc.scalar.activation(sq[:], qf[:], AF.Square)
                H = P // 2
                sw = rp.tile([P, STILE], F32, tag="sw")
                nc.sync.dma_start(sw[0:H, :], qf[H:P, :])
                nc.sync.dma_start(sw[H:P, :], qf[0:H, :])
                return (qf, sw), sq

            def stage_B(oi, sq):
                """ms = sum-and-broadcast matmul; 1/rms = exp(-.5 ln ms) on ACT.
                For k the c0 factor rides the Exp bias; result is a [128, 512]
                bf16 broadcast tile ready for a plain DVE multiply."""
                is_q = oi != "k"
                wm = winvq_sb if is_q else winvk_sb
                ms = psl.tile([P, STILE], F32, tag="ms")
                nc.tensor.matmul(ms[:], wm[:], sq[:], start=True, stop=True)
                lns = t1p.tile([P, STILE], F32, tag="lns")
                nc.scalar.activation(lns[:], ms[:], AF.Ln,
                                     bias=eps_sb[:], scale=1.0 / DK)
                pb = pbp.tile([P, STILE], BF16, tag="pb")
                if is_q:
                    nc.scalar.activation(pb[:], lns[:], AF.Exp, scale=-0.5)
                else:
                    nc.scalar.activation(pb[:], lns[:], AF.Exp,
                                         bias=lnc0_sb[:], scale=-0.5)
                return pb

            def stage_C(oi, qfsw, pb, ssl):
                """RoPE + normalize (and fold c0/rms_k for k)"""
                qf, sw = qfsw
                is_q = oi != "k"
                ta = rp.tile([P, STILE], F32, tag="ta")
                tb = rp.tile([P, STILE], F32, tag="tb")
                nc.vector.tensor_mul(ta[:], qf[:], taba_sb[:, ssl])
                nc.vector.tensor_mul(tb[:], sw[:], tabb_sb[:, ssl])
                rt = rp.tile([P, STILE], F32, tag="rope")
                nc.vector.tensor_add(rt[:], ta[:], tb[:])
                dst = qhat[:, oi, ssl] if is_q else khat[:, ssl]
                nc.vector.tensor_mul(dst, rt[:], pb[:])

            def stage_V(xt, st):
                """v as [dk, s] big matmuls, then PE-transpose to [s%128, c]"""
                ptv = ps1.tile([P, STILE], F32, tag="proj", name=f"ptv{st}")
                for ch in range(NC_CHUNKS):
                    nc.tensor.matmul(ptv[:], wv_sb[:, ch, :], xt[:, ch, :],
                                     start=(ch == 0), stop=(ch == NC_CHUNKS - 1))
                vtmp = t1p.tile([P, STILE], BF16, tag="vtmp")
                nc.scalar.activation(vtmp[:], ptv[:], AF.Identity)
                return vtmp

            def stage_Vt(vtmp, st):
                vps = pst.tile([P, STILE], BF16, tag="vt")
                for sb in range(4):
                    nc.tensor.transpose(vps[:, bass.ts(sb, DK)],
                                        vtmp[:, bass.ts(sb, P)], id_sb[:])
                nc.vector.tensor_copy(vsb[:, st * 4 : st * 4 + 4, :],
                                      vps[:].rearrange("p (a c) -> p a c", a=4))

            for st in range(N_STILE):
                if st == 0:
                    xt = xt0
                elif st == 1:
                    xt = xt1
                else:
                    xt = xp.tile([P, NC_CHUNKS, STILE], BF16, tag="xt")
                    for g4 in range(0, NC_CHUNKS, 4):
                        sl4 = slice(g4, g4 + 4)
                        nc.sync.dma_start(xt[:, sl4, :],
                                          xt4[st, sl4].rearrange("c p s -> p c s"))
                ssl = bass.ts(st, STILE)

                # 2-deep pipeline over the 6 outputs (4 q heads, v, k):
                # A(h) proj; B(h) after A(h+1); C(h) after A(h+2) — the PE
                # never waits on the ACT rms chain.
                qf_, pb_ = {}, {}
                qf_[0], sq0 = stage_A(0, xt, first=(st == 0))
                qf_[1], sq1 = stage_A(1, xt)
                pb_[0] = stage_B(0, sq0)
                qf_[2], sq2 = stage_A(2, xt)
                pb_[1] = stage_B(1, sq1)
                stage_C(0, qf_[0], pb_[0], ssl)
                qf_[3], sq3 = stage_A(3, xt)
                pb_[2] = stage_B(2, sq2)
                stage_C(1, qf_[1], pb_[1], ssl)
                vtmp = stage_V(xt, st)
                pb_[3] = stage_B(3, sq3)
                stage_C(2, qf_[2], pb_[2], ssl)
                stage_Vt(vtmp, st)
                kf, sqk = stage_A("k", xt)
                pbk = stage_B("k", sqk)
                stage_C(3, qf_[3], pb_[3], ssl)
                stage_C("k", kf, pbk, ssl)

        # ---------------- Phase 2: attention ----------------
        nc.sync.dma_start(mask_sb[:], maskt[:])
        nc.sync.dma_start(wo_sb[:], wo.rearrange("f p e -> p f e"))
        nc.sync.dma_start(bo_sb[:], boeff[:])

        with ExitStack() as ph2:
            pp = ph2.enter_context(tc.tile_pool(name="pp", bufs=5))
            pp2 = ph2.enter_context(tc.tile_pool(name="pp2", bufs=2))
            lp = ph2.enter_context(tc.tile_pool(name="lp", bufs=2))
            rlp = ph2.enter_context(tc.tile_pool(name="rlp", bufs=2))
            ypb = ph2.enter_context(tc.tile_pool(name="ypb", bufs=4))
            psst = ph2.enter_context(tc.tile_pool(name="psst", bufs=2, space="PSUM"))
            psao = ph2.enter_context(tc.tile_pool(name="psao", bufs=2, space="PSUM"))
            psli = ph2.enter_context(tc.tile_pool(name="psli", bufs=1, space="PSUM"))
            psy = ph2.enter_context(tc.tile_pool(name="psy", bufs=1, space="PSUM"))

            yT_v = yT.rearrange("(eb p) s -> eb p s", p=P)
            chunk_queue = []

            def emit_chunk(pool_y, pool_sb):
                """one out-projection chunk (eb, st): 4 PE matmuls + DVE bias
                + DMA. Interleaved into the ACT-bound attention stream."""
                eb, st = chunk_queue.pop(0)
                yps = pool_y.tile([P, STILE], F32, tag="y", name=f"y{eb}_{st}")
                for fc in range(NH_LOC):
                    nc.tensor.matmul(yps[:], wo_sb[:, fc, bass.ts(eb, P)],
                                     aon[:, fc, bass.ts(st, STILE)],
                                     start=(fc == 0), stop=(fc == NH_LOC - 1))
                y_sb = pool_sb.tile([P, STILE], BF16, tag="ysb")
                if eb % 2 == 0:  # split bias-adds across DVE and ACT
                    nc.vector.tensor_scalar_add(y_sb[:], yps[:],
                                                bo_sb[:, eb : eb + 1])
                else:
                    nc.scalar.activation(y_sb[:], yps[:], AF.Identity,
                                         bias=bo_sb[:, eb : eb + 1])
                nc.sync.dma_start(yT_v[eb][:, bass.ts(st, STILE)], y_sb[:])

            def emit_tail_ln(pl2, ao_ps, h, it):
                """Ln first: frees the single denominator psum bank early"""
                lnl = lp.tile([P, STILE], F32, tag="lnl")
                nc.scalar.activation(lnl[:], pl2[:], AF.Ln)
                return (lnl, ao_ps, h, it)

            def emit_tail_rest(lnl, ao_ps, h, it):
                rlb_sb = rlp.tile([P, STILE], BF16, tag="rlbs")
                nc.scalar.activation(rlb_sb[:], lnl[:], AF.Exp, scale=-1.0)
                nc.vector.tensor_mul(aon[:, h, bass.ts(it, STILE)],
                                     ao_ps[:], rlb_sb[:])
                if h == NH_LOC - 1:
                    # aon(*, it) complete: its out-projection chunks are ready
                    chunk_queue.extend((eb, it) for eb in range(NJB))

            def lo_of(jb, it):
                t = jb - 4 * it
                return P * t if t > 0 else 0

            # flat list of j-block-pair jobs, software-pipelined one pair
            # ahead; it-major so aon(*, it) completes in it order and the
            # st-major phase 3 never waits on the last softmax tail
            jobs = []
            for it in range(N_STILE):
                njb = 4 * it + 4
                for h in range(NH_LOC):
                    for pr in range(njb // 2):
                        jobs.append((h, it, pr, njb))

            st_tiles = {}

            def emit_scores_pair(k):
                """half 0 valid at [lo0:512], half 1 PACKED at [512:1024-lo1]
                so the pair is one contiguous region for a single ACT exp."""
                h, it, pr, njb = jobs[k]
                stp = psst.tile([P, 2 * STILE], F32, tag="st")
                for half in (0, 1):
                    jb = 2 * pr + half
                    lo = lo_of(jb, it)
                    o = STILE * half + (lo if half == 0 else 0)
                    nc.tensor.matmul(
                        stp[:, o : o + STILE - lo],
                        khat[:, bass.ts(jb, P)],
                        qhat[:, h, bass.ds(it * STILE + lo, STILE - lo)],
                        start=True, stop=True)
                st_tiles[k] = stp

            acc = {}
            pending = None
            pending_ln = None
            emit_scores_pair(0)
            for k, (h, it, pr, njb) in enumerate(jobs):
                if pr == 0:
                    ao_t = psao.tile([P, STILE], F32, tag="ao", name=f"ao{h}_{it}")
                    pl_t = psli.tile([P, STILE], F32, tag="li", name=f"li{h}_{it}")
                    acc[(h, it)] = (ao_t, pl_t)
                ao_ps, pl2 = acc.pop((h, it)) if pr == njb // 2 - 1 else acc[(h, it)]
                stp = st_tiles.pop(k)
                lo0 = lo_of(2 * pr, it)
                lo1 = lo_of(2 * pr + 1, it)
                ptile = pp.tile([P, 2 * STILE], BF16, tag="p")
                nc.scalar.activation(ptile[:, lo0 : 2 * STILE - lo1],
                                     stp[:, lo0 : 2 * STILE - lo1], AF.Exp)
                # tail split across jobs: Ln right away (frees the single
                # denominator bank), Exp+apply after the NEXT exp so AV
                # matmuls never wait on tail ACTs
                if pr == 0 and pending is not None:
                    pending_ln = emit_tail_ln(*pending)
                    pending = None
                elif pr == 1 and pending_ln is not None:
                    emit_tail_rest(*pending_ln)
                    pending_ln = None
                for half in (0, 1):
                    jb = 2 * pr + half
                    t = jb - 4 * it
                    lo = lo_of(jb, it)
                    if t >= 0:
                        o = STILE * half + (lo if half == 0 else 0)
                        nc.vector.tensor_mul(ptile[:, o : o + P],
                                             ptile[:, o : o + P], mask_sb[:])
                if k + 1 < len(jobs):
                    emit_scores_pair(k + 1)
                full_pair = (2 * pr + 1) < 4 * it  # both halves full-width
                for half in (0, 1):
                    jb = 2 * pr + half
                    lo = lo_of(jb, it)
                    csl = slice(lo, STILE)
                    o = STILE * half + (lo if half == 0 else 0)
                    psl_ = slice(o, o + STILE - lo)
                    nc.tensor.matmul(ao_ps[:, csl], vsb[:, jb, :], ptile[:, psl_],
                                     start=(jb == 0), stop=(jb == njb - 1))
                if not full_pair and pr == 2 * it:
                    # first diagonal pair: seed the diagonal P-sum on DVE
                    dacc = pp2.tile([P, STILE], BF16, tag="da")
                    nc.vector.tensor_copy(dacc[:], ptile[:, 0:STILE])
                    w1 = STILE - lo1
                    nc.vector.tensor_add(dacc[:, lo1:], dacc[:, lo1:],
                                         ptile[:, STILE : STILE + w1])
                elif not full_pair:
                    # second diagonal pair: finish the sum, fold in the quad
                    # total; ONE denominator matmul per (h, it)
                    nc.vector.tensor_add(dacc[:, lo0:], dacc[:, lo0:],
                                         ptile[:, lo0:STILE])
                    nc.vector.tensor_add(dacc[:, lo1:], dacc[:, lo1:],
                                         ptile[:, STILE : STILE + STILE - lo1])
                    if it > 0:
                        nc.vector.tensor_add(dacc[:], dacc[:], ptot[:])
                    nc.tensor.matmul(pl2[:], ones_mat[:], dacc[:],
                                     start=True, stop=True)
                if full_pair:
                    # partial-sum P on the idle DVE so the denominator matmul
                    # streams 1 column set per 4 j-blocks instead of 4
                    tp = pp2.tile([P, STILE], BF16, tag="tp")
                    nc.vector.tensor_add(tp[:], ptile[:, 0:STILE],
                                         ptile[:, STILE : 2 * STILE])
                    if pr % 2 == 0:
                        quad_prev = tp
                    else:
                        tq = pp2.tile([P, STILE], BF16, tag="tq")
                        nc.vector.tensor_add(tq[:], quad_prev[:], tp[:])
                        if 2 * pr + 1 == 3:
                            ptot = tq
                        else:
                            pt2 = pp2.tile([P, STILE], BF16, tag="pt")
                            nc.vector.tensor_add(pt2[:], ptot[:], tq[:])
                            ptot = pt2
                if chunk_queue:
                    emit_chunk(psy, ypb)
                if pr == njb // 2 - 1:
                    pending = (pl2, ao_ps, h, it)
            emit_tail_rest(*emit_tail_ln(*pending))

        # ------- Phase 3 rump: chunks not absorbed into phase 2 -------
        with ExitStack() as ph3:
            yp3 = ph3.enter_context(tc.tile_pool(name="yp3", bufs=6))
            psy3 = ph3.enter_context(tc.tile_pool(name="psy3", bufs=4, space="PSUM"))
            while chunk_queue:
                emit_chunk(psy3, yp3)

    if split:
        _split_excess_waits(nc)
    return nc


_PERM = np.concatenate([np.arange(0, DK, 2), np.arange(1, DK, 2)])  # de-interleave


def _prep_inputs(x, Wq, bq, Wk, bk, Wv, bv, Wo, bo, q_norm_w, k_norm_w):
    """Build the 8 per-core input maps. Core c -> (b = c // 4, g = c % 4)."""
    def bf(a):
        return np.ascontiguousarray(a).astype(_BF)

    wq_p = q_norm_w[_PERM].astype(np.float32)
    wk_p = k_norm_w[_PERM].astype(np.float32)
    with np.errstate(divide="ignore"):
        winvq = np.where(wq_p != 0, 1.0 / np.maximum(wq_p * wq_p, 1e-30), 0.0)
        winvk = np.where(wk_p != 0, 1.0 / np.maximum(wk_p * wk_p, 1e-30), 0.0)

    inv_freq = 1.0 / (10000.0 ** (np.arange(0, DK, 2, dtype=np.float32) / np.float32(DK)))
    freqs = np.arange(S, dtype=np.float32)[:, None] * inv_freq[None, :]
    cosT = np.cos(freqs).T.astype(np.float32)  # [64, S]
    sinT = np.sin(freqs).T.astype(np.float32)
    taba = np.vstack([cosT, cosT]).copy()          # [128, S]
    tabb = np.vstack([-sinT, sinT]).copy()

    pj = np.arange(P)[:, None]
    fi = np.arange(P)[None, :]
    maskt = (pj <= fi).astype(np.float32)  # [128, 128] lower-tri (j <= i)

    xt4_b = []
    for b in range(2):
        xt = x[b].T.astype(np.float32)  # [d, s]
        xt4_b.append(bf(xt.reshape(NC_CHUNKS, P, N_STILE, STILE).transpose(2, 0, 1, 3)))

    winvq_m = np.repeat(winvq[:, None], P, axis=1)
    winvk_m = np.repeat(winvk[:, None], P, axis=1)
    id128 = np.eye(P, dtype=np.float32)

    in_maps = []
    for core in range(8):
        b, g = divmod(core, NH_LOC)
        hsl = slice(g * NH_LOC * DK, (g + 1) * NH_LOC * DK)
        ksl = slice(g * DK, (g + 1) * DK)

        wq_blk = Wq[hsl].astype(np.float32).copy()  # [512, d]
        # per-head de-interleave permutation + fold q_norm_w
        wq_blk = wq_blk.reshape(NH_LOC, DK, D)[:, _PERM, :] * wq_p[None, :, None]
        wq_t = wq_blk.reshape(NH_LOC * DK, D).T.reshape(NC_CHUNKS, P, NH_LOC * DK)

        wk_blk = Wk[ksl].astype(np.float32)[_PERM, :] * wk_p[:, None]
        wk_t = wk_blk.T.reshape(NC_CHUNKS, P, DK)
        wv_t = Wv[ksl].astype(np.float32).T.reshape(NC_CHUNKS, P, DK)
        wo_t = Wo[:, hsl].astype(np.float32).T.reshape(NH_LOC, P, D)

        bq_blk = bq[hsl].astype(np.float32).reshape(NH_LOC, DK)[:, _PERM].T.copy()  # [128, 4]
        bk_blk = bk[ksl].astype(np.float32)[_PERM][:, None].copy()
        if g == 0:
            boeff = bo.astype(np.float32).reshape(NJB, P).T.copy()
        else:
            boeff = np.zeros((P, NJB), np.float32)

        in_maps.append({
            "xt4": xt4_b[b],
            "wq": bf(wq_t), "wk": bf(wk_t), "wv": bf(wv_t), "wo": bf(wo_t),
            "winvq": bf(winvq_m), "winvk": bf(winvk_m), "id128": bf(id128),
            "taba": bf(taba), "tabb": bf(tabb),
            "maskt": bf(maskt),
            "bq": np.ascontiguousarray(bq_blk), "bk": bk_blk, "boeff": boeff,
        })
    return in_maps


_CACHED = {}


def _get_program():
    if "nc" not in _CACHED:
        _CACHED["nc"] = _build_program()
    return _CACHED["nc"]


def kernel(x, Wq, bq, Wk, bk, Wv, bv, Wo, bo, q_norm_w, k_norm_w, _trace=False, _tmpdir=None):
    x = np.asarray(x, np.float32)
    args = [np.asarray(a, np.float32) for a in
            (Wq, bq, Wk, bk, Wv, bv, Wo, bo, q_norm_w, k_norm_w)]
    Wq, bq, Wk, bk, Wv, bv, Wo, bo, q_norm_w, k_norm_w = args

    nc = _get_program()
    in_maps = _prep_inputs(x, Wq, bq, Wk, bk, Wv, bv, Wo, bo, q_norm_w, k_norm_w)
    res = run_bass_kernel_spmd(nc, in_maps, list(range(8)), trace=_trace, tmpdir=_tmpdir)

    out = np.zeros((2, S, D), np.float32)
    for core in range(8):
        b = core // 4
        out[b] += res.results[core]["yT"].T.astype(np.float32)
    # v-bias enters only via softmax-weighted average (weights sum to 1):
    if np.any(bv):
        out += (np.repeat(bv.reshape(4, DK), 4, axis=0).reshape(D) @ Wo.T)[None, None, :]
    kernel._last_result = res
    return out
